# revision 1
# baseline (speedup 1.0000x reference)
"""DeepSeek-MoE layer as a Bass/Tile kernel on 8 Trainium2 NeuronCores.

Strategy (expert-parallel, dense):
  - Each core holds 4 of the 32 routed experts (weights sharded on host) plus
    a 1/8 tensor-parallel slice (352 rows, padded to 384) of the shared expert's
    intermediate dimension.
  - The router (gate matmul fp32 + sigmoid + grouped top-6) is replicated on
    every core; it is exact w.r.t. the fp32 reference selection.
  - Expert GEMMs run in bf16 (fp32 PSUM accumulation). The per-token combine
    weights are folded into the activations before the down-projection, so the
    down-proj GEMMs of all local experts + shared slice accumulate directly in
    PSUM per 128-row output chunk.
  - Partial outputs [H, T] are summed across cores with an on-device
    ReduceScatter; each core returns its 256-row H-slice, the host concatenates
    and transposes.

kernel(**inputs) takes the full unsharded inputs and returns the full output.
"""

import numpy as np
import ml_dtypes

# ---- model dims (hardcoded per problem spec) ----
T = 512          # tokens
H = 2048         # hidden
E = 32           # routed experts
G = 8            # groups
GS = E // G      # experts per group = 4
TKG = 4          # top-k groups
TOPK = 6         # experts per token
I = 1408         # moe intermediate
SCALE = 2.5
NCORES = 8
EL = E // NCORES       # local experts = 4
KC = H // 128          # 16 h-chunks
IM = I // 128          # 11 i-chunks per expert
MH = KC                # 16 output h-chunks
NT = T // 128          # 4 token tiles
M13 = 2 * IM           # 22 w13 m-chunks
SH = (2 * I) // NCORES     # shared slice = 352
SHP = 384                  # padded shared slice
SHC = SHP // 128           # 3 chunks
BIG = 1.0e5

_CACHE = {}


def _build_nc():
    import concourse.bacc as bacc
    import concourse.mybir as mybir
    import concourse.tile as tile
    from concourse.masks import make_identity

    F32 = mybir.dt.float32
    BF16 = mybir.dt.bfloat16
    ALU = mybir.AluOpType
    AFT = mybir.ActivationFunctionType
    AX = mybir.AxisListType

    nc = bacc.Bacc("TRN2", target_bir_lowering=False, debug=False,
                   enable_asserts=True, num_devices=NCORES)

    xT_d = nc.dram_tensor("xT", [H, T], F32, kind="ExternalInput").ap()
    xTb_d = nc.dram_tensor("xTb", [H, T], BF16, kind="ExternalInput").ap()
    gw_d = nc.dram_tensor("gw", [128, KC, E], F32, kind="ExternalInput").ap()
    cb_d = nc.dram_tensor("cb", [1, E], F32, kind="ExternalInput").ap()
    sel_d = nc.dram_tensor("sel", [E, EL * 128], F32, kind="ExternalInput").ap()
    w13_d = nc.dram_tensor("w13t", [EL, M13, 128, KC, 128], BF16, kind="ExternalInput").ap()
    w2_d = nc.dram_tensor("w2t", [EL, MH, 128, IM, 128], BF16, kind="ExternalInput").ap()
    sg_d = nc.dram_tensor("sgt", [SHC, 128, KC, 128], BF16, kind="ExternalInput").ap()
    su_d = nc.dram_tensor("sut", [SHC, 128, KC, 128], BF16, kind="ExternalInput").ap()
    swd_d = nc.dram_tensor("swdt", [MH, 128, SHC, 128], BF16, kind="ExternalInput").ap()
    out_d = nc.dram_tensor("out", [H // NCORES, T], F32, kind="ExternalOutput").ap()

    with tile.TileContext(nc) as tc:
        with tc.tile_pool(name="per", bufs=1) as per, \
             tc.tile_pool(name="rt", bufs=1) as rt, \
             tc.tile_pool(name="acts", bufs=1) as acts, \
             tc.tile_pool(name="wstream", bufs=4) as wstream, \
             tc.tile_pool(name="w2stream", bufs=6) as w2stream, \
             tc.tile_pool(name="ep", bufs=3) as ep, \
             tc.tile_pool(name="ps", bufs=4, space="PSUM") as ps, \
             tc.tile_pool(name="ps2", bufs=2, space="PSUM") as ps2, \
             tc.tile_pool(name="dram", bufs=1, space="DRAM") as dram:

            # ---------- loads ----------
            xT_bf = per.tile([128, KC, T], BF16)
            for k in range(KC):
                nc.sync.dma_start(xT_bf[:, k, :], xTb_d[k * 128:(k + 1) * 128, :])
            xT = per.tile([128, KC, T], F32)
            for k in range(KC):
                nc.sync.dma_start(xT[:, k, :], xT_d[k * 128:(k + 1) * 128, :])
            gw = per.tile([128, KC, E], F32)
            nc.sync.dma_start(gw[:], gw_d[:])
            cb_row = per.tile([1, E], F32)
            nc.sync.dma_start(cb_row[:], cb_d[:])
            sel_sb = per.tile([E, EL * 128], F32)
            nc.sync.dma_start(sel_sb[:], sel_d[:])
            ones_row = per.tile([1, 128], F32)
            nc.vector.memset(ones_row[:], 1.0)
            ident = per.tile([128, 128], F32)
            make_identity(nc, ident)

            # corr_bias broadcast to [128, E] via K=1 matmul
            ps_cb = ps2.tile([128, E], F32, tag="small")
            nc.tensor.matmul(ps_cb[:], ones_row[:], cb_row[:], start=True, stop=True)
            cb_bc = rt.tile([128, E], F32)
            nc.vector.tensor_copy(cb_bc[:], ps_cb[:])

            # ---------- gate GEMM (fp32) ----------
            scores = rt.tile([128, NT, E], F32)
            for i in range(NT):
                pl = ps2.tile([128, E], F32, tag="small")
                for k in range(KC):
                    nc.tensor.matmul(pl[:], xT[:, k, i * 128:(i + 1) * 128],
                                     gw[:, k, :], start=(k == 0), stop=(k == KC - 1))
                nc.scalar.activation(scores[:, i, :], pl[:], AFT.Sigmoid)

            # ---------- grouped top-k routing (exact fp32) ----------
            sfc = rt.tile([128, NT, E], F32)
            nc.vector.tensor_tensor(sfc[:], scores[:],
                                    cb_bc[:, None, :].to_broadcast([128, NT, E]), ALU.add)
            sfc_g = sfc[:].rearrange("p n (g s) -> p n g s", s=GS)
            v = [sfc_g[:, :, :, j] for j in range(GS)]
            grp = rt.tile([128, NT, G], F32)
            gtmp = rt.tile([128, NT, G], F32)
            first = True
            for (a, b) in [(0, 1), (2, 3), (0, 2), (0, 3), (1, 2), (1, 3)]:
                nc.vector.tensor_add(gtmp[:], v[a], v[b])
                if first:
                    nc.vector.tensor_copy(grp[:], gtmp[:])
                    first = False
                else:
                    nc.vector.tensor_max(grp[:], grp[:], gtmp[:])

            gmask = rt.tile([128, NT, G], F32)
            nc.vector.memset(gmask[:], 0.0)
            gm = rt.tile([128, NT], F32)
            gism = rt.tile([128, NT, G], F32)
            for _ in range(TKG):
                nc.vector.tensor_reduce(gm[:], grp[:], AX.X, ALU.max)
                nc.vector.tensor_tensor(gism[:], grp[:],
                                        gm[:, :, None].to_broadcast([128, NT, G]), ALU.is_equal)
                nc.vector.tensor_add(gmask[:], gmask[:], gism[:])
                nc.vector.scalar_tensor_tensor(grp[:], gism[:], -BIG, grp[:], ALU.mult, ALU.add)

            ngmask = rt.tile([128, NT, G], F32)
            nc.vector.tensor_scalar(ngmask[:], gmask[:], -1.0, 1.0, ALU.mult, ALU.add)
            msfc = rt.tile([128, NT, E], F32)
            msfc_g = msfc[:].rearrange("p n (g s) -> p n g s", s=GS)
            nc.vector.scalar_tensor_tensor(
                msfc_g, ngmask[:, :, :, None].to_broadcast([128, NT, G, GS]), -BIG,
                sfc_g, ALU.mult, ALU.add)

            sel = rt.tile([128, NT, E], F32)
            nc.vector.memset(sel[:], 0.0)
            km = rt.tile([128, NT], F32)
            kism = rt.tile([128, NT, E], F32)
            for _ in range(TOPK):
                nc.vector.tensor_reduce(km[:], msfc[:], AX.X, ALU.max)
                nc.vector.tensor_tensor(kism[:], msfc[:],
                                        km[:, :, None].to_broadcast([128, NT, E]), ALU.is_equal)
                nc.vector.tensor_add(sel[:], sel[:], kism[:])
                nc.vector.scalar_tensor_tensor(msfc[:], kism[:], -BIG, msfc[:], ALU.mult, ALU.add)

            wsel = rt.tile([128, NT, E], F32)
            nc.vector.tensor_mul(wsel[:], scores[:], sel[:])
            den = rt.tile([128, NT], F32)
            nc.vector.tensor_reduce(den[:], wsel[:], AX.X, ALU.add)
            rin = rt.tile([128, NT], F32)
            nc.vector.reciprocal(rin[:], den[:])
            nc.vector.tensor_scalar_mul(rin[:], rin[:], float(SCALE))
            cw = rt.tile([128, NT, E], F32)
            nc.vector.tensor_tensor(cw[:], wsel[:],
                                    rin[:, :, None].to_broadcast([128, NT, E]), ALU.mult)

            # transpose cw -> cwT [E, T], then per-local-expert broadcast rows
            ps_cwT = ps2.tile([E, T], F32, tag="scratch")
            for i in range(NT):
                nc.tensor.transpose(ps_cwT[:, i * 128:(i + 1) * 128], cw[:, i, :], ident[:])
            cwT = rt.tile([E, T], F32)
            nc.vector.tensor_copy(cwT[:], ps_cwT[:])

            cw_bc = per.tile([128, EL, T], F32)
            for le in range(EL):
                ps_b = ps2.tile([128, T], F32, tag="scratch")
                nc.tensor.matmul(ps_b[:], sel_sb[:, le * 128:(le + 1) * 128], cwT[:],
                                 start=True, stop=True)
                nc.vector.tensor_copy(cw_bc[:, le, :], ps_b[:])

            # ---------- expert GEMM1 + silu + combine-weight fold ----------
            act = acts.tile([128, EL, IM, T], BF16)
            for le in range(EL):
                for im in range(IM):
                    wg = wstream.tile([128, KC, 128], BF16, tag="w13")
                    nc.sync.dma_start(wg[:], w13_d[le, im])
                    wu = wstream.tile([128, KC, 128], BF16, tag="w13")
                    nc.sync.dma_start(wu[:], w13_d[le, IM + im])
                    pg = ps.tile([128, T], F32, tag="mm")
                    pu = ps.tile([128, T], F32, tag="mm")
                    for k in range(KC):
                        nc.tensor.matmul(pg[:], wg[:, k, :], xT_bf[:, k, :],
                                         start=(k == 0), stop=(k == KC - 1))
                    for k in range(KC):
                        nc.tensor.matmul(pu[:], wu[:, k, :], xT_bf[:, k, :],
                                         start=(k == 0), stop=(k == KC - 1))
                    sil = ep.tile([128, T], F32, tag="sil")
                    nc.scalar.activation(sil[:], pg[:], AFT.Sigmoid)
                    tm = ep.tile([128, T], F32, tag="tm")
                    nc.vector.tensor_mul(tm[:], sil[:], pg[:])
                    nc.vector.tensor_mul(tm[:], tm[:], pu[:])
                    nc.vector.tensor_mul(act[:, le, im, :], tm[:], cw_bc[:, le, :])

            # shared expert slice GEMM1
            act_sh = acts.tile([128, SHC, T], BF16)
            for im in range(SHC):
                wg = wstream.tile([128, KC, 128], BF16, tag="w13")
                nc.sync.dma_start(wg[:], sg_d[im])
                wu = wstream.tile([128, KC, 128], BF16, tag="w13")
                nc.sync.dma_start(wu[:], su_d[im])
                pg = ps.tile([128, T], F32, tag="mm")
                pu = ps.tile([128, T], F32, tag="mm")
                for k in range(KC):
                    nc.tensor.matmul(pg[:], wg[:, k, :], xT_bf[:, k, :],
                                     start=(k == 0), stop=(k == KC - 1))
                for k in range(KC):
                    nc.tensor.matmul(pu[:], wu[:, k, :], xT_bf[:, k, :],
                                     start=(k == 0), stop=(k == KC - 1))
                sil = ep.tile([128, T], F32, tag="sil")
                nc.scalar.activation(sil[:], pg[:], AFT.Sigmoid)
                tm = ep.tile([128, T], F32, tag="tm")
                nc.vector.tensor_mul(tm[:], sil[:], pg[:])
                nc.vector.tensor_mul(act_sh[:, im, :], tm[:], pu[:])

            # ---------- GEMM2: accumulate all local experts + shared ----------
            rs_in0 = dram.tile([H // 2, T], F32)
            rs_in1 = dram.tile([H // 2, T], F32)
            rs_out0 = dram.tile([H // (2 * NCORES), T], F32)
            rs_out1 = dram.tile([H // (2 * NCORES), T], F32)
            for mh in range(MH):
                po = ps.tile([128, T], F32, tag="mm")
                for le in range(EL):
                    w2b = w2stream.tile([128, IM, 128], BF16, tag="w2")
                    nc.sync.dma_start(w2b[:], w2_d[le, mh])
                    for ki in range(IM):
                        nc.tensor.matmul(po[:], w2b[:, ki, :], act[:, le, ki, :],
                                         start=(le == 0 and ki == 0), stop=False)
                swdb = w2stream.tile([128, SHC, 128], BF16, tag="swd")
                nc.sync.dma_start(swdb[:], swd_d[mh])
                for ki in range(SHC):
                    nc.tensor.matmul(po[:], swdb[:, ki, :], act_sh[:, ki, :],
                                     start=False, stop=(ki == SHC - 1))
                om = ep.tile([128, T], F32, tag="om")
                nc.vector.tensor_copy(om[:], po[:])
                half, row = divmod(mh, MH // 2)
                rs_tgt = rs_in1 if half else rs_in0
                nc.sync.dma_start(rs_tgt[row * 128:(row + 1) * 128, :], om[:])
                if mh == MH // 2 - 1:
                    nc.gpsimd.collective_compute(
                        "ReduceScatter", ALU.add,
                        replica_groups=[list(range(NCORES))],
                        ins=[rs_in0.opt()], outs=[rs_out0.opt()])

            nc.gpsimd.collective_compute(
                "ReduceScatter", ALU.add,
                replica_groups=[list(range(NCORES))],
                ins=[rs_in1.opt()], outs=[rs_out1.opt()])
            nc.sync.dma_start(out_d[0:128, :], rs_out0[:])
            nc.sync.dma_start(out_d[128:256, :], rs_out1[:])

    nc.compile()
    return nc


def _prep_in_maps(inputs):
    bf16 = ml_dtypes.bfloat16
    x = np.ascontiguousarray(np.asarray(inputs["hidden_states"], dtype=np.float32))
    gate_w = np.ascontiguousarray(np.asarray(inputs["gate_w"], dtype=np.float32))
    corr_bias = np.asarray(inputs["corr_bias"], dtype=np.float32).reshape(1, E)
    w13 = np.asarray(inputs["w13"])
    w2 = np.asarray(inputs["w2"])
    sgu = np.asarray(inputs["shared_w_gu"])
    swd = np.asarray(inputs["shared_w_down"])

    xT = np.ascontiguousarray(x.T)                      # [H, T]
    xTb = xT.astype(bf16)
    gw_t = np.ascontiguousarray(gate_w.reshape(KC, 128, E).transpose(1, 0, 2))
    w13_bf = w13.astype(bf16)                           # [E, H, 2I]
    w2_bf = w2.astype(bf16)                             # [E, I, H]
    sgu_bf = sgu.astype(bf16)                           # [H, 2*2816]
    swd_bf = swd.astype(bf16)                           # [2816, H]

    in_maps = []
    for c in range(NCORES):
        e0 = c * EL
        # w13 lhsT tiles: [e, m, p(h%128), k(h//128), f(d%128)]
        w13t = np.ascontiguousarray(
            w13_bf[e0:e0 + EL].reshape(EL, KC, 128, M13, 128).transpose(0, 3, 2, 1, 4))
        # w2 lhsT tiles: [e, mh, p(i%128), ki(i//128), f(h%128)]
        w2t = np.ascontiguousarray(
            w2_bf[e0:e0 + EL].reshape(EL, IM, 128, MH, 128).transpose(0, 3, 2, 1, 4))
        # shared gate/up slices padded to SHP rows of intermediate
        g_sl = sgu_bf[:, c * SH:(c + 1) * SH]           # [H, 352]
        u_sl = sgu_bf[:, 2 * I + c * SH:2 * I + (c + 1) * SH]
        g_pad = np.zeros((H, SHP), dtype=bf16); g_pad[:, :SH] = g_sl
        u_pad = np.zeros((H, SHP), dtype=bf16); u_pad[:, :SH] = u_sl
        sgt = np.ascontiguousarray(
            g_pad.reshape(KC, 128, SHC, 128).transpose(2, 1, 0, 3))
        sut = np.ascontiguousarray(
            u_pad.reshape(KC, 128, SHC, 128).transpose(2, 1, 0, 3))
        d_pad = np.zeros((SHP, H), dtype=bf16); d_pad[:SH] = swd_bf[c * SH:(c + 1) * SH]
        swdt = np.ascontiguousarray(
            d_pad.reshape(SHC, 128, MH, 128).transpose(2, 1, 0, 3))
        # selector: sel[k, le*128 + j] = 1 iff k == e0 + le
        sel = np.zeros((E, EL * 128), dtype=np.float32)
        for le in range(EL):
            sel[e0 + le, le * 128:(le + 1) * 128] = 1.0
        in_maps.append({
            "xT": xT, "xTb": xTb, "gw": gw_t, "cb": corr_bias, "sel": sel,
            "w13t": w13t, "w2t": w2t, "sgt": sgt, "sut": sut, "swdt": swdt,
        })
    return in_maps


def _get_nc():
    if "nc" not in _CACHE:
        _CACHE["nc"] = _build_nc()
    return _CACHE["nc"]


def _run(inputs, trace=False, tmpdir=None):
    from concourse.bass_utils import run_bass_kernel_spmd
    nc = _get_nc()
    in_maps = _prep_in_maps(inputs)
    res = run_bass_kernel_spmd(nc, in_maps, core_ids=list(range(NCORES)),
                               trace=trace, tmpdir=tmpdir)
    h0 = np.concatenate([res.results[c]["out"][0:128] for c in range(NCORES)], axis=0)
    h1 = np.concatenate([res.results[c]["out"][128:256] for c in range(NCORES)], axis=0)
    outT = np.concatenate([h0, h1], axis=0)
    out = np.ascontiguousarray(outT.T).astype(np.float32)
    return out, res


def kernel(**inputs) -> np.ndarray:
    try:
        r = _run_sparse(inputs, trace=False)
    except Exception:
        r = None
    if r is not None:
        return r[0]
    out, _ = _run(inputs, trace=False)
    return out


# ======================= token-sparse variant =======================
# Host gathers each expert's routed tokens (dispatch-by-topk_ids sharding)
# into 128-wide chunks; per-core uniform slot profile keeps the program SPMD.
# Device still computes routing/cw; host-gathered extras are zeroed by cw.
PROF = (3, 2, 1, 1)          # chunks per expert slot
JW = 128 * sum(PROF)         # gathered width per core = 896
OFFS = (0, 384, 640, 768)    # column offsets per slot
NJ = sum(PROF)               # 7 chunks


def _host_routing(x, gate_w, corr_bias):
    """Replicates reference._grouped_topk selection in fp64-backed numpy."""
    logits = x.astype(np.float64) @ gate_w.astype(np.float64)
    scores = 1.0 / (1.0 + np.exp(-logits))
    sfc = scores + corr_bias[None, :].astype(np.float64)
    grp = sfc.reshape(T, G, GS)
    top2 = np.sort(grp, -1)[..., -2:].sum(-1)
    gidx = np.argsort(-top2, -1)[:, :TKG]
    gmask = np.zeros((T, G))
    np.put_along_axis(gmask, gidx, 1.0, 1)
    masked = np.where(np.repeat(gmask, GS, 1) > 0, sfc, -np.inf)
    kidx = np.argsort(-masked, -1)[:, :TOPK]
    return kidx  # [T, TOPK]


def _sparse_plan(topk_ids):
    """Assign experts to 8x(3,2,1,1) slots; None if loads don't fit."""
    loads = np.bincount(topk_ids.ravel(), minlength=E)
    chunks = np.maximum(1, -(-loads // 128))  # >=1 chunk per expert
    if chunks.max() > max(PROF):
        return None
    order = np.argsort(-chunks)
    slots = []  # (capacity, core, slot)
    for c in range(NCORES):
        for s, cap in enumerate(PROF):
            slots.append([cap, c, s])
    slots.sort(key=lambda t: -t[0])
    assign = {}
    used = set()
    for e in order:
        placed = False
        for i, (cap, c, s) in enumerate(slots):
            if i in used or cap < chunks[e]:
                continue
            assign[int(e)] = (c, s)
            used.add(i)
            placed = True
            break
        if not placed:
            return None
    tok_lists = [list(np.where((topk_ids == e).any(1))[0]) for e in range(E)]
    return assign, tok_lists


def _build_nc_sparse():
    import concourse.bacc as bacc
    import concourse.mybir as mybir
    import concourse.tile as tile
    from concourse.masks import make_identity

    F32 = mybir.dt.float32
    BF16 = mybir.dt.bfloat16
    ALU = mybir.AluOpType
    AFT = mybir.ActivationFunctionType
    AX = mybir.AxisListType

    nc = bacc.Bacc("TRN2", target_bir_lowering=False, debug=False,
                   enable_asserts=True, num_devices=NCORES)

    xT_d = nc.dram_tensor("xT", [H, T], F32, kind="ExternalInput").ap()
    xTb_d = nc.dram_tensor("xTb", [H, T], BF16, kind="ExternalInput").ap()
    gw_d = nc.dram_tensor("gw", [128, KC, E], F32, kind="ExternalInput").ap()
    cb_d = nc.dram_tensor("cb", [1, E], F32, kind="ExternalInput").ap()
    selc_d = nc.dram_tensor("selc", [E, NJ * 128], F32, kind="ExternalInput").ap()
    xTg_d = nc.dram_tensor("xTg", [128, KC, JW], BF16, kind="ExternalInput").ap()
    smat_d = nc.dram_tensor("smat", [128, NJ, T], BF16, kind="ExternalInput").ap()
    w13_d = nc.dram_tensor("w13t", [EL, M13, 128, KC, 128], BF16, kind="ExternalInput").ap()
    w2_d = nc.dram_tensor("w2t", [EL, MH, 128, IM, 128], BF16, kind="ExternalInput").ap()
    sg_d = nc.dram_tensor("sgt", [SHC, 128, KC, 128], BF16, kind="ExternalInput").ap()
    su_d = nc.dram_tensor("sut", [SHC, 128, KC, 128], BF16, kind="ExternalInput").ap()
    swdr_d = nc.dram_tensor("swdr", [SHC, 128, H], BF16, kind="ExternalInput").ap()
    out_d = nc.dram_tensor("out", [T // NCORES, H], F32, kind="ExternalOutput").ap()

    with tile.TileContext(nc) as tc:
        with tc.tile_pool(name="per", bufs=1) as per, \
             tc.tile_pool(name="rt", bufs=1) as rt, \
             tc.tile_pool(name="acts", bufs=1) as acts, \
             tc.tile_pool(name="wstream", bufs=6) as wstream, \
             tc.tile_pool(name="w2stream", bufs=6) as w2stream, \
             tc.tile_pool(name="ep", bufs=3) as ep, \
             tc.tile_pool(name="ps", bufs=4, space="PSUM") as ps, \
             tc.tile_pool(name="ps2", bufs=4, space="PSUM") as ps2, \
             tc.tile_pool(name="dram", bufs=1, space="DRAM") as dram:

            # ---------- loads (gate-critical first) ----------
            gw = per.tile([128, KC, E], F32)
            nc.sync.dma_start(gw[:], gw_d[:])
            cb_row = per.tile([1, E], F32)
            nc.sync.dma_start(cb_row[:], cb_d[:])
            xT_bf = per.tile([128, KC, T], BF16)
            xTg = per.tile([128, KC, JW], BF16)
            for k in range(KC):
                nc.gpsimd.dma_start(xTg[:, k, :], xTg_d[:, k, :])
            selc = per.tile([E, NJ * 128], F32)
            nc.gpsimd.dma_start(selc[:], selc_d[:])
            smat = per.tile([128, NJ, T], BF16)
            nc.gpsimd.dma_start(smat[:], smat_d[:])
            ones_row = per.tile([1, 128], F32)
            nc.vector.memset(ones_row[:], 1.0)
            ident = per.tile([128, 128], F32)
            make_identity(nc, ident)
            ident_bf = per.tile([128, 128], BF16)
            nc.vector.tensor_copy(ident_bf[:], ident[:])

            ps_cb = ps2.tile([128, E], F32, tag="scratch")
            nc.tensor.matmul(ps_cb[:], ones_row[:], cb_row[:], start=True, stop=True)
            cb_bc = rt.tile([128, E], F32)
            nc.vector.tensor_copy(cb_bc[:], ps_cb[:])

            # ---------- gate GEMM + routing (fp32 x streamed per chunk) ----------
            scores = rt.tile([128, NT, E], F32)
            pls = [ps2.tile([128, E], F32, tag="scratch", name=f"pl{i}") for i in range(NT)]
            for k in range(KC):
                xtk = wstream.tile([128, T], F32, tag="xtk", bufs=2)
                nc.sync.dma_start(xtk[:], xT_d[k * 128:(k + 1) * 128, :])
                for i in range(NT):
                    nc.tensor.matmul(pls[i][:], xtk[:, i * 128:(i + 1) * 128],
                                     gw[:, k, :], start=(k == 0), stop=(k == KC - 1))
            for i in range(NT):
                nc.scalar.activation(scores[:, i, :], pls[i][:], AFT.Sigmoid)

            sfc = rt.tile([128, NT, E], F32)
            nc.vector.tensor_tensor(sfc[:], scores[:],
                                    cb_bc[:, None, :].to_broadcast([128, NT, E]), ALU.add)
            sfc_g = sfc[:].rearrange("p n (g s) -> p n g s", s=GS)
            v = [sfc_g[:, :, :, j] for j in range(GS)]
            grp = rt.tile([128, NT, G], F32)
            gtmp = rt.tile([128, NT, G], F32)
            first = True
            for (a, b) in [(0, 1), (2, 3), (0, 2), (0, 3), (1, 2), (1, 3)]:
                nc.vector.tensor_add(gtmp[:], v[a], v[b])
                if first:
                    nc.vector.tensor_copy(grp[:], gtmp[:])
                    first = False
                else:
                    nc.vector.tensor_max(grp[:], grp[:], gtmp[:])

            gmask = rt.tile([128, NT, G], F32)
            nc.vector.memset(gmask[:], 0.0)
            gm = rt.tile([128, NT], F32)
            gism = rt.tile([128, NT, G], F32)
            for _ in range(TKG):
                nc.vector.tensor_reduce(gm[:], grp[:], AX.X, ALU.max)
                nc.vector.tensor_tensor(gism[:], grp[:],
                                        gm[:, :, None].to_broadcast([128, NT, G]), ALU.is_equal)
                nc.vector.tensor_add(gmask[:], gmask[:], gism[:])
                nc.vector.scalar_tensor_tensor(grp[:], gism[:], -BIG, grp[:], ALU.mult, ALU.add)

            ngmask = rt.tile([128, NT, G], F32)
            nc.vector.tensor_scalar(ngmask[:], gmask[:], -1.0, 1.0, ALU.mult, ALU.add)
            msfc = rt.tile([128, NT, E], F32)
            msfc_g = msfc[:].rearrange("p n (g s) -> p n g s", s=GS)
            nc.vector.scalar_tensor_tensor(
                msfc_g, ngmask[:, :, :, None].to_broadcast([128, NT, G, GS]), -BIG,
                sfc_g, ALU.mult, ALU.add)

            sel = rt.tile([128, NT, E], F32)
            nc.vector.memset(sel[:], 0.0)
            km = rt.tile([128, NT], F32)
            kism = rt.tile([128, NT, E], F32)
            for _ in range(TOPK):
                nc.vector.tensor_reduce(km[:], msfc[:], AX.X, ALU.max)
                nc.vector.tensor_tensor(kism[:], msfc[:],
                                        km[:, :, None].to_broadcast([128, NT, E]), ALU.is_equal)
                nc.vector.tensor_add(sel[:], sel[:], kism[:])
                nc.vector.scalar_tensor_tensor(msfc[:], kism[:], -BIG, msfc[:], ALU.mult, ALU.add)

            wsel = rt.tile([128, NT, E], F32)
            nc.vector.tensor_mul(wsel[:], scores[:], sel[:])
            den = rt.tile([128, NT], F32)
            nc.vector.tensor_reduce(den[:], wsel[:], AX.X, ALU.add)
            rin = rt.tile([128, NT], F32)
            nc.vector.reciprocal(rin[:], den[:])
            nc.vector.tensor_scalar_mul(rin[:], rin[:], float(SCALE))
            cw = rt.tile([128, NT, E], F32)
            nc.vector.tensor_tensor(cw[:], wsel[:],
                                    rin[:, :, None].to_broadcast([128, NT, E]), ALU.mult)

            ps_cwT = ps2.tile([E, T], F32, tag="scratch")
            for i in range(NT):
                nc.tensor.transpose(ps_cwT[:, i * 128:(i + 1) * 128], cw[:, i, :], ident[:])
            cwT = rt.tile([E, T], F32)
            nc.vector.tensor_copy(cwT[:], ps_cwT[:])

            # per-chunk weighted scatter matrices: S'_j = smat_j * cw_row(expert_of_chunk_j)
            sprime = per.tile([128, NJ, T], BF16)
            for j in range(NJ):
                ps_b = ps2.tile([128, T], F32, tag="scratch")
                nc.tensor.matmul(ps_b[:], selc[:, j * 128:(j + 1) * 128], cwT[:],
                                 start=True, stop=True)
                nc.vector.tensor_mul(sprime[:, j, :], smat[:, j, :], ps_b[:])

            # ---------- sparse expert GEMM1 ----------
            # wide and narrow slots interleaved so narrow slots' LDWEIGHTS
            # hide behind wide slots' matmul streaming
            act = acts.tile([128, IM, JW], BF16)
            for pair in ((0, 2), (1, 3)):
                for im in range(IM):
                    for le in pair:
                        cwid = PROF[le] * 128
                        off = OFFS[le]
                        wg = wstream.tile([128, KC, 128], BF16, tag="w13")
                        nc.sync.dma_start(wg[:], w13_d[le, im])
                        wu = wstream.tile([128, KC, 128], BF16, tag="w13")
                        nc.sync.dma_start(wu[:], w13_d[le, IM + im])
                        pg = ps.tile([128, T], F32, tag="mm")
                        pu = ps.tile([128, T], F32, tag="mm")
                        for k in range(KC):
                            nc.tensor.matmul(pg[:, :cwid], wg[:, k, :],
                                             xTg[:, k, off:off + cwid],
                                             start=(k == 0), stop=(k == KC - 1))
                        for k in range(KC):
                            nc.tensor.matmul(pu[:, :cwid], wu[:, k, :],
                                             xTg[:, k, off:off + cwid],
                                             start=(k == 0), stop=(k == KC - 1))
                        sil = ep.tile([128, T], F32, tag="sil")
                        nc.scalar.activation(sil[:, :cwid], pg[:, :cwid], AFT.Sigmoid)
                        tm = ep.tile([128, T], F32, tag="tm")
                        nc.vector.tensor_mul(tm[:, :cwid], sil[:, :cwid], pg[:, :cwid])
                        nc.vector.tensor_mul(tm[:, :cwid], tm[:, :cwid], pu[:, :cwid])
                        nc.vector.tensor_copy(act[:, im, off:off + cwid], tm[:, :cwid])

            # shared expert slice GEMM1 (dense over all tokens)
            for k in range(KC):
                nc.sync.dma_start(xT_bf[:, k, :], xTb_d[k * 128:(k + 1) * 128, :])
            act_sh = acts.tile([128, SHC, T], BF16)
            for im in range(SHC):
                wg = wstream.tile([128, KC, 128], BF16, tag="w13")
                nc.sync.dma_start(wg[:], sg_d[im])
                wu = wstream.tile([128, KC, 128], BF16, tag="w13")
                nc.sync.dma_start(wu[:], su_d[im])
                pg = ps.tile([128, T], F32, tag="mm")
                pu = ps.tile([128, T], F32, tag="mm")
                for k in range(KC):
                    nc.tensor.matmul(pg[:], wg[:, k, :], xT_bf[:, k, :],
                                     start=(k == 0), stop=(k == KC - 1))
                for k in range(KC):
                    nc.tensor.matmul(pu[:], wu[:, k, :], xT_bf[:, k, :],
                                     start=(k == 0), stop=(k == KC - 1))
                sil = ep.tile([128, T], F32, tag="sil")
                nc.scalar.activation(sil[:], pg[:], AFT.Sigmoid)
                tm = ep.tile([128, T], F32, tag="tm")
                nc.vector.tensor_mul(tm[:], sil[:], pg[:])
                nc.vector.tensor_mul(act_sh[:, im, :], tm[:], pu[:])

            # ---------- sparse GEMM2 (mh-outer) + transpose to [token, h] ----------
            swdr = per.tile([128, SHC, H], BF16)
            for ki in range(SHC):
                nc.sync.dma_start(swdr[:, ki, :], swdr_d[ki])
            rs_in0 = dram.tile([T, H // 2], F32)
            rs_in1 = dram.tile([T, H // 2], F32)
            rs_out0 = dram.tile([T // NCORES, H // 2], F32)
            rs_out1 = dram.tile([T // NCORES, H // 2], F32)
            eo = acts.tile([128, NJ, MH, 128], BF16)

            def combine_hh(hh):
                for tt in range(NT):
                    po = ps.tile([128, 512], F32, tag="mm", name=f"poc{hh}_{tt}")
                    for j in range(NJ):
                        nc.tensor.matmul(po[:], sprime[:, j, tt * 128:(tt + 1) * 128],
                                         eo[:, j, hh * 4:(hh + 1) * 4, :],
                                         start=(j == 0), stop=False)
                    for ki in range(SHC):
                        nc.tensor.matmul(po[:], act_sh[:, ki, tt * 128:(tt + 1) * 128],
                                         swdr[:, ki, hh * 512:(hh + 1) * 512],
                                         start=False, stop=(ki == SHC - 1))
                    om = ep.tile([128, 512], F32, tag="om")
                    nc.vector.tensor_copy(om[:], po[:])
                    half, col = divmod(hh, (H // 512) // 2)
                    rs_tgt = rs_in1 if half else rs_in0
                    nc.sync.dma_start(
                        rs_tgt[tt * 128:(tt + 1) * 128, col * 512:(col + 1) * 512], om[:])

            for mh in range(MH):
                for le in range(EL):
                    cwid = PROF[le] * 128
                    off = OFFS[le]
                    jbase = off // 128
                    w2b = w2stream.tile([128, IM, 128], BF16, tag="w2")
                    nc.sync.dma_start(w2b[:], w2_d[le, mh])
                    po = ps.tile([128, T], F32, tag="mm")
                    for ki in range(IM):
                        nc.tensor.matmul(po[:, :cwid], w2b[:, ki, :], act[:, ki, off:off + cwid],
                                         start=(ki == 0), stop=(ki == IM - 1))
                    eoT = ep.tile([128, T], BF16, tag="eoT")
                    nc.vector.tensor_copy(eoT[:, :cwid], po[:, :cwid])
                    for jj in range(PROF[le]):
                        ptr = ps2.tile([128, 128], BF16, tag="scratch")
                        nc.tensor.transpose(ptr[:], eoT[:, jj * 128:(jj + 1) * 128], ident_bf[:])
                        nc.vector.tensor_copy(eo[:, jbase + jj, mh, :], ptr[:])
                # after each 4-mh block (one 512-wide h chunk), combine + maybe RS
                if mh % 4 == 3:
                    combine_hh(mh // 4)
                if mh == 7:
                    nc.gpsimd.collective_compute(
                        "ReduceScatter", ALU.add,
                        replica_groups=[list(range(NCORES))],
                        ins=[rs_in0.opt()], outs=[rs_out0.opt()])
            nc.gpsimd.dma_start(out_d[:, 0:H // 2], rs_out0[:])
            nc.gpsimd.collective_compute(
                "ReduceScatter", ALU.add,
                replica_groups=[list(range(NCORES))],
                ins=[rs_in1.opt()], outs=[rs_out1.opt()])
            nc.gpsimd.dma_start(out_d[:, H // 2:H], rs_out1[:])

    nc.compile()
    return nc


def _prep_sparse_in_maps(inputs, assign, tok_lists):
    bf16 = ml_dtypes.bfloat16
    x = np.ascontiguousarray(np.asarray(inputs["hidden_states"], dtype=np.float32))
    gate_w = np.ascontiguousarray(np.asarray(inputs["gate_w"], dtype=np.float32))
    corr_bias = np.asarray(inputs["corr_bias"], dtype=np.float32).reshape(1, E)
    w13 = np.asarray(inputs["w13"])
    w2 = np.asarray(inputs["w2"])
    sgu = np.asarray(inputs["shared_w_gu"])
    swd = np.asarray(inputs["shared_w_down"])

    xT = np.ascontiguousarray(x.T)
    xTb = xT.astype(bf16)
    gw_t = np.ascontiguousarray(gate_w.reshape(KC, 128, E).transpose(1, 0, 2))
    w13_bf = w13.astype(bf16)
    w2_bf = w2.astype(bf16)
    sgu_bf = sgu.astype(bf16)
    swd_bf = swd.astype(bf16)

    slot_expert = {}  # (core, slot) -> expert
    for e, (c, s) in assign.items():
        slot_expert[(c, s)] = e

    in_maps = []
    for c in range(NCORES):
        # per-slot experts; unused slots get expert index None (zero weights)
        w13t = np.zeros((EL, M13, 128, KC, 128), dtype=bf16)
        w2t = np.zeros((EL, MH, 128, IM, 128), dtype=bf16)
        xTg = np.zeros((128, KC, JW), dtype=bf16)
        smat = np.zeros((128, NJ, T), dtype=bf16)
        selc = np.zeros((E, NJ * 128), dtype=np.float32)
        for s in range(EL):
            e = slot_expert.get((c, s))
            if e is None:
                continue
            w13t[s] = w13_bf[e].reshape(KC, 128, M13, 128).transpose(2, 1, 0, 3)
            w2t[s] = w2_bf[e].reshape(IM, 128, MH, 128).transpose(2, 1, 0, 3)
            toks = tok_lists[e]
            jbase = OFFS[s] // 128
            gx = np.zeros((PROF[s] * 128, H), dtype=np.float32)
            n = len(toks)
            gx[:n] = x[toks]
            xTg[:, :, OFFS[s]:OFFS[s] + PROF[s] * 128] = (
                gx.T.reshape(KC, 128, PROF[s] * 128).transpose(1, 0, 2).astype(bf16))
            for jj in range(PROF[s]):
                lo = jj * 128
                seg = toks[lo:lo + 128]
                for u, t in enumerate(seg):
                    smat[u, jbase + jj, t] = 1.0
                selc[e, (jbase + jj) * 128:(jbase + jj + 1) * 128] = 1.0
        g_sl = sgu_bf[:, c * SH:(c + 1) * SH]
        u_sl = sgu_bf[:, 2 * I + c * SH:2 * I + (c + 1) * SH]
        g_pad = np.zeros((H, SHP), dtype=bf16); g_pad[:, :SH] = g_sl
        u_pad = np.zeros((H, SHP), dtype=bf16); u_pad[:, :SH] = u_sl
        sgt = np.ascontiguousarray(g_pad.reshape(KC, 128, SHC, 128).transpose(2, 1, 0, 3))
        sut = np.ascontiguousarray(u_pad.reshape(KC, 128, SHC, 128).transpose(2, 1, 0, 3))
        d_pad = np.zeros((SHP, H), dtype=bf16); d_pad[:SH] = swd_bf[c * SH:(c + 1) * SH]
        swdr = np.ascontiguousarray(d_pad.reshape(SHC, 128, H))
        in_maps.append({
            "xT": xT, "xTb": xTb, "gw": gw_t, "cb": corr_bias,
            "selc": selc, "xTg": np.ascontiguousarray(xTg),
            "smat": np.ascontiguousarray(smat),
            "w13t": np.ascontiguousarray(w13t), "w2t": np.ascontiguousarray(w2t),
            "sgt": sgt, "sut": sut, "swdr": swdr,
        })
    return in_maps


def _run_sparse(inputs, trace=False, tmpdir=None):
    from concourse.bass_utils import run_bass_kernel_spmd
    x = np.asarray(inputs["hidden_states"], dtype=np.float32)
    gate_w = np.asarray(inputs["gate_w"], dtype=np.float32)
    corr_bias = np.asarray(inputs["corr_bias"], dtype=np.float32)
    plan = _sparse_plan(_host_routing(x, gate_w, corr_bias))
    if plan is None:
        return None
    assign, tok_lists = plan
    if "nc_sparse" not in _CACHE:
        _CACHE["nc_sparse"] = _build_nc_sparse()
    nc = _CACHE["nc_sparse"]
    in_maps = _prep_sparse_in_maps(inputs, assign, tok_lists)
    res = run_bass_kernel_spmd(nc, in_maps, core_ids=list(range(NCORES)),
                               trace=trace, tmpdir=tmpdir)
    out = np.concatenate([res.results[c]["out"] for c in range(NCORES)], axis=0)
    return np.ascontiguousarray(out).astype(np.float32), res

